# revision 1
# baseline (speedup 1.0000x reference)
"""Trainium2 Bass kernel for CustomAttentionWithPE.

Reference computation (B=2, S=2048, H=16, Dh=64, D=1024):
    qkv = hs @ W_qkv + b_qkv ; split to q,k,v per head
    q,k = RoPE(q), RoPE(k)
    out = softmax(q k^T / 8) v   (no mask)
    return concat_heads(out) @ W_o + b_o

Sharding: 8 cores -> (batch b = core//4, head-quad g = core%4, heads 4g..4g+3).
Each core computes partial = attn(heads of g, batch b) @ W_o[rows of g]
for its batch; host sums the 4 partials per batch and adds the bias terms
(b_o and the V-bias contribution b_v @ W_o; softmax rows sum to 1 so the
V bias contributes exactly b_v @ W_o per token).

Device pipeline per core: all matmul operands bf16 (fp32 PSUM accum; fp32
runs 4 cyc/row on the PE and fp32r trips the PE power throttle to ~50%
util), RoPE math in fp32 with a single bf16 rounding on write.
  phase 1, per 512-token stripe: xT tiles DMA'd bf16; Q^T/K^T via PE
  (PSUM->SBUF drain on ScalarE), V natural via PE (drain on DVE, +ones
  cols so PV also yields the softmax denominator Z), RoPE on DVE+GpSimd.
  phase 2, per (q-stripe, head-pair): per k-tile one combined-heads
  score PSUM tile [128, 2x512] (two PE matmuls, row groups 0/64), ONE
  exp on ScalarE [128,1024] PSUM->SBUF bf16 (scale=0.125 folds the
  1/sqrt(dh)), two PV accumulations over k-tiles into [66, 512] PSUM.
  1/Z via DVE reciprocal, broadcast across partitions by a tiny
  bf16 PE matmul; normalized att written bf16. Output projection reuses
  the score PSUM pool; drains to DRAM via DVE copies.
"""

import math
from contextlib import ExitStack

import numpy as np

import concourse.bass as bass
import concourse.mybir as mybir
import concourse.tile as tile
from concourse.bass_utils import run_bass_kernel_spmd

F32 = mybir.dt.float32
BF = mybir.dt.bfloat16
AF = mybir.ActivationFunctionType

B, S, D = 2, 2048, 1024
NH, HD = 16, 64
ROPE_BASE = 10000.0
N_CORES = 8
HPC = 4  # heads per core
DLOC = HPC * HD  # 256 local head dims per core


def _split_sync_waits(nc, maxw=1):
    """This container's walrus rejects >1-2 SyncWaits per instruction
    ("Too many sync wait commands"). Move excess waits onto NoOps."""
    for f in nc.m.functions:
        for blk in f.blocks:
            new_instructions = []
            for ins in blk.instructions:
                si = getattr(ins, "sync_info", None)
                if si is not None and si.on_wait and len(si.on_wait) > maxw:
                    waits = list(si.on_wait)
                    extra, keep = waits[:-maxw], waits[-maxw:]
                    si.on_wait = keep
                    for i in range(0, len(extra), maxw):
                        nop = mybir.InstNoOp(
                            name=nc.get_next_instruction_name(),
                            engine=ins.engine,
                            sync_info=mybir.SyncInfo(
                                on_wait=extra[i : i + maxw], on_update=[]
                            ),
                        )
                        nc.register_instruction(nop, overwrite=True)
                        new_instructions.append(nop)
                new_instructions.append(ins)
            blk.instructions[:] = new_instructions


def build_attention_nc(seq=S, add_qk_bias=False):
    """One SPMD program; per-core data differs only through inputs."""
    nc = bass.Bass()
    NT = seq // 512  # 512-token stripes
    KT = seq // 128  # k tiles
    NCH = D // 128  # contraction chunks over d_model

    xT = nc.dram_tensor("xT", [D, seq], BF, kind="ExternalInput")
    wq = nc.dram_tensor("wq", [D, DLOC], BF, kind="ExternalInput")
    wk = nc.dram_tensor("wk", [D, DLOC], BF, kind="ExternalInput")
    wv = nc.dram_tensor("wv", [D, DLOC], BF, kind="ExternalInput")
    wo = nc.dram_tensor("wo", [DLOC, D], BF, kind="ExternalInput")
    cosT = nc.dram_tensor("cosT", [HD, seq], F32, kind="ExternalInput")
    sinT = nc.dram_tensor("sinT", [HD, seq], F32, kind="ExternalInput")
    bqk = nc.dram_tensor("bqk", [2, DLOC], F32, kind="ExternalInput")
    out = nc.dram_tensor("out", [seq, D], F32, kind="ExternalOutput")

    mm = nc.tensor.matmul

    with tile.TileContext(nc) as tc, ExitStack() as ctx:
        consts = ctx.enter_context(tc.tile_pool(name="consts", bufs=1))
        # weights as [128, chunk, cols]; row d = c*128 + p
        wq_sb = consts.tile([128, NCH, DLOC], BF)
        nc.sync.dma_start(out=wq_sb, in_=wq.rearrange("(c p) m -> p c m", p=128))
        wk_sb = consts.tile([128, NCH, DLOC], BF)
        nc.sync.dma_start(out=wk_sb, in_=wk.rearrange("(c p) m -> p c m", p=128))
        wv_sb = consts.tile([128, NCH, DLOC], BF)
        nc.sync.dma_start(out=wv_sb, in_=wv.rearrange("(c p) m -> p c m", p=128))
        # cos/sin rows duplicated for the two heads of a pair; wo is not
        # needed until phase 2 so its DMA is issued last
        cs_sb = consts.tile([128, seq], F32)
        nc.sync.dma_start(out=cs_sb[0:HD, :], in_=cosT[:])
        nc.sync.dma_start(out=cs_sb[HD:128, :], in_=cosT[:])
        sn_sb = consts.tile([128, seq], F32)
        nc.sync.dma_start(out=sn_sb[0:HD, :], in_=sinT[:])
        nc.sync.dma_start(out=sn_sb[HD:128, :], in_=sinT[:])
        wo_sb = consts.tile([128, 2, D], BF)
        nc.sync.dma_start(out=wo_sb, in_=wo.rearrange("(c p) m -> p c m", p=128))
        # [2, HD] stationary for the 1/Z partition-broadcast: row0=1, row1=0
        ones_sb = consts.tile([2, HD], BF)
        nc.vector.memset(ones_sb, 0.0)
        nc.vector.memset(ones_sb[0:1, :], 1.0)
        if add_qk_bias:
            bqk_sb = consts.tile([128, 2, 2], F32)
            nc.sync.dma_start(
                out=bqk_sb, in_=bqk.rearrange("b (h p) -> p b h", p=128)
            )

        # long-lived activation tensors
        acts = ctx.enter_context(tc.tile_pool(name="acts", bufs=1))
        qtr = acts.tile([128, 2, seq], BF)  # RoPE'd Q^T, head pairs
        ktr = acts.tile([128, 2, seq], BF)
        v_sb = acts.tile([128, KT, HPC, HD + 2], BF)  # V natural + ones cols
        att = acts.tile([128, 2, seq], BF)  # normalized attn out ^T
        # both pad cols 1.0: PSUM rows 64,65 then both hold Z, keeping the
        # 2-partition reciprocal finite (row65's ones_sb weight is 0)
        nc.vector.memset(v_sb[:, :, :, HD : HD + 2], 1.0)

        # ---------------- phase 1: QKV projection + RoPE -------------
        with ExitStack() as p1:
            xpool = p1.enter_context(tc.tile_pool(name="xT", bufs=NCH + 2))
            qraw_pool = p1.enter_context(tc.tile_pool(name="qraw", bufs=1))
            ps1 = p1.enter_context(
                tc.tile_pool(name="ps1", bufs=2, space="PSUM")
            )
            rope_tmp = p1.enter_context(tc.tile_pool(name="ropetmp", bufs=2))

            qt_raw = qraw_pool.tile([128, 2, seq], F32)
            kt_raw = qraw_pool.tile([128, 2, seq], F32)

            for nt in range(NT):
                cs = slice(nt * 512, nt * 512 + 512)
                xts = []
                for c in range(NCH):
                    xt = xpool.tile([128, 512], BF, tag="xt")
                    nc.sync.dma_start(
                        out=xt, in_=xT[c * 128 : (c + 1) * 128, cs]
                    )
                    xts.append(xt)
                for hp in range(2):
                    for dst, w in ((qt_raw, wq_sb), (kt_raw, wk_sb)):
                        ps = ps1.tile([128, 512], F32, tag="qk")
                        for c in range(NCH):
                            mm(
                                ps,
                                w[:, c, hp * 128 : hp * 128 + 128],
                                xts[c],
                                start=(c == 0),
                                stop=(c == NCH - 1),
                            )
                        # drain on ScalarE (idle during phase 1)
                        nc.scalar.copy(dst[:, hp, cs], ps)
                # V natural: out [128 tokens, 256 vcols]
                for tt in range(4):
                    ps = ps1.tile([128, DLOC], F32, tag="v")
                    for c in range(NCH):
                        mm(
                            ps,
                            xts[c][:, tt * 128 : tt * 128 + 128],
                            wv_sb[:, c, :],
                            start=(c == 0),
                            stop=(c == NCH - 1),
                        )
                    kt_idx = nt * 4 + tt
                    for h in range(HPC):
                        nc.vector.tensor_copy(
                            v_sb[:, kt_idx, h, 0:HD], ps[:, h * HD : (h + 1) * HD]
                        )

                if add_qk_bias:
                    for hp in range(2):
                        nc.vector.tensor_scalar_add(
                            qt_raw[:, hp, cs], qt_raw[:, hp, cs],
                            bqk_sb[:, 0, hp : hp + 1],
                        )
                        nc.vector.tensor_scalar_add(
                            kt_raw[:, hp, cs], kt_raw[:, hp, cs],
                            bqk_sb[:, 1, hp : hp + 1],
                        )

                # RoPE for this stripe: dst = raw*cos + rot(raw)*sin
                # rot rows (per 64-block): [0:32] = -raw[32:64], [32:64] = +raw[0:32]
                for raw, dst in ((qt_raw, qtr), (kt_raw, ktr)):
                    rot = rope_tmp.tile([128, 2, 512], F32, tag="rot")
                    for base in (0, 64):
                        nc.vector.tensor_scalar_mul(
                            rot[base : base + 32, :, :],
                            raw[base + 32 : base + 64, :, cs],
                            -1.0,
                        )
                        nc.vector.tensor_copy(
                            rot[base + 32 : base + 64, :, :],
                            raw[base : base + 32, :, cs],
                        )
                    for hp in range(2):
                        tmp = rope_tmp.tile([128, 512], F32, tag="tmp")
                        nc.vector.tensor_mul(tmp, raw[:, hp, cs], cs_sb[:, cs])
                        rs = rope_tmp.tile([128, 512], F32, tag="rs")
                        nc.vector.tensor_mul(rs, rot[:, hp, :], sn_sb[:, cs])
                        nc.vector.tensor_add(dst[:, hp, cs], tmp, rs)

        # ---------------- phase 2: attention + output projection -----
        with ExitStack() as p2:
            # PSUM budget (16KB/partition): sc 2x4KB + pv 2x2KB + zb 2KB = 14KB
            # (WO reuses the sc pool's banks)
            ps_sc = p2.enter_context(
                tc.tile_pool(name="ps_sc", bufs=2, space="PSUM")
            )
            ps_pv = p2.enter_context(
                tc.tile_pool(name="ps_pv", bufs=2, space="PSUM")
            )
            ps_zb = p2.enter_context(
                tc.tile_pool(name="ps_zb", bufs=1, space="PSUM")
            )
            slab = p2.enter_context(tc.tile_pool(name="slab", bufs=3))
            npool = p2.enter_context(tc.tile_pool(name="norm", bufs=4))
            opool = p2.enter_context(tc.tile_pool(name="ostage", bufs=3))

            for qt in range(NT):
                qs = slice(qt * 512, qt * 512 + 512)
                for hp in range(2):
                    pv = [
                        ps_pv.tile([128, 512], F32, tag="pv", name="pv0"),
                        ps_pv.tile([128, 512], F32, tag="pv", name="pv1"),
                    ]
                    for kt_idx in range(KT):
                        # combined-heads score tile: cols (head, q)
                        sc = ps_sc.tile([128, 2, 512], F32, tag="sc")
                        for h in range(2):
                            hb = h * 64
                            mm(
                                sc[:, h, :],
                                ktr[
                                    hb : hb + 64,
                                    hp,
                                    kt_idx * 128 : kt_idx * 128 + 128,
                                ],
                                qtr[hb : hb + 64, hp, qs],
                                start=True,
                                stop=True,
                            )
                        pt = slab.tile([128, 2, 512], BF, tag="pt")
                        nc.scalar.activation(pt, sc, AF.Exp, scale=0.125)
                        for h in range(2):
                            mm(
                                pv[h][0 : HD + 2, :],
                                v_sb[:, kt_idx, hp * 2 + h, :],
                                pt[:, h, :],
                                start=(kt_idx == 0),
                                stop=(kt_idx == KT - 1),
                                skip_group_check=True,
                            )
                    # normalize: att[h-rows, hp, qs] = pv[0:64] * (1/Z bcast)
                    for h in range(2):
                        hb = h * 64
                        o_sb = npool.tile([128, 512], F32, tag="osb")
                        nc.vector.tensor_copy(o_sb[hb : hb + 64, :], pv[h][0:HD, :])
                        # 1/Z = exp(-ln Z) on ScalarE: cheaper than the
                        # multi-pass DVE reciprocal and off the DVE path
                        lnz = npool.tile([2, 512], F32, tag="lnz")
                        nc.scalar.activation(lnz, pv[h][HD : HD + 2, :], AF.Ln)
                        zrow = npool.tile([2, 512], BF, tag="z")
                        nc.scalar.activation(zrow, lnz, AF.Exp, scale=-1.0)
                        zb = ps_zb.tile([128, 512], F32, tag="zb")
                        mm(
                            zb[hb : hb + 64, :],
                            ones_sb[:, :],
                            zrow[0:2, :],
                            start=True,
                            stop=True,
                            tile_position=(0, hb),
                        )
                        nc.vector.tensor_mul(
                            att[hb : hb + 64, hp, qs],
                            o_sb[hb : hb + 64, :],
                            zb[hb : hb + 64, :],
                        )
                # output projection for this 512-token stripe
                for tt in range(4):
                    tok = qt * 512 + tt * 128
                    ps = ps_sc.tile([128, 2, 512], F32, tag="sc", name="wops")
                    for nh in range(2):
                        for hp in range(2):
                            mm(
                                ps[:, nh, :],
                                att[:, hp, tok : tok + 128],
                                wo_sb[:, hp, nh * 512 : nh * 512 + 512],
                                start=(hp == 0),
                                stop=(hp == 1),
                            )
                    o_out = opool.tile([128, 2, 512], F32, tag="oo")
                    nc.vector.tensor_copy(o_out, ps)
                    nc.sync.dma_start(
                        out=out[tok : tok + 128, :], in_=o_out
                    )

    _split_sync_waits(nc, maxw=1)
    return nc


_NC_CACHE = {}


def _rope_cos_sin(seq):
    inv_freq = 1.0 / (
        ROPE_BASE ** (np.arange(0, HD, 2, dtype=np.float32) / HD)
    )
    pos = np.arange(seq, dtype=np.float32)
    freqs = pos[:, None] * inv_freq[None, :]  # [seq, 32]
    emb = np.concatenate([freqs, freqs], axis=-1)  # [seq, 64]
    return np.cos(emb).astype(np.float32), np.sin(emb).astype(np.float32)


def _bf16(a):
    import ml_dtypes

    return np.ascontiguousarray(np.asarray(a, dtype=np.float32)).astype(
        ml_dtypes.bfloat16
    )


def kernel(hidden_states, W_qkv, b_qkv, W_o, b_o):
    hs = np.asarray(hidden_states, dtype=np.float32)
    W_qkv = np.asarray(W_qkv, dtype=np.float32)
    b_qkv = np.asarray(b_qkv, dtype=np.float32)
    W_o = np.asarray(W_o, dtype=np.float32)
    b_o = np.asarray(b_o, dtype=np.float32)
    b, seq, d = hs.shape

    bq, bk, bv = b_qkv[:D], b_qkv[D : 2 * D], b_qkv[2 * D :]
    add_qk_bias = bool(np.any(bq) or np.any(bk))

    key = (seq, add_qk_bias)
    if key not in _NC_CACHE:
        _NC_CACHE[key] = build_attention_nc(seq, add_qk_bias)
    nc = _NC_CACHE[key]

    cos, sin = _rope_cos_sin(seq)
    cosT = np.ascontiguousarray(cos.T)
    sinT = np.ascontiguousarray(sin.T)

    in_maps = []
    for core in range(N_CORES):
        bb, g = core // 4, core % 4
        cols = slice(g * DLOC, (g + 1) * DLOC)
        in_maps.append(
            {
                "xT": _bf16(hs[bb].T),
                "wq": _bf16(W_qkv[:, cols]),
                "wk": _bf16(W_qkv[:, 1024:][:, cols]),
                "wv": _bf16(W_qkv[:, 2048:][:, cols]),
                "wo": _bf16(W_o[cols, :]),
                "cosT": cosT,
                "sinT": sinT,
                "bqk": np.stack([bq[cols], bk[cols]]),
            }
        )

    res = run_bass_kernel_spmd(nc, in_maps, list(range(N_CORES)))
    parts = [res.results[c]["out"] for c in range(N_CORES)]
    outv = np.stack(
        [parts[0] + parts[1] + parts[2] + parts[3],
         parts[4] + parts[5] + parts[6] + parts[7]]
    )
    outv += b_o[None, None, :] + (bv @ W_o)[None, None, :]
    return outv.astype(np.float32)



# revision 6
# speedup vs baseline: 1.1238x; 1.1238x over previous
"""Trainium2 Bass kernel for CustomAttentionWithPE.

Reference computation (B=2, S=2048, H=16, Dh=64, D=1024):
    qkv = hs @ W_qkv + b_qkv ; split to q,k,v per head
    q,k = RoPE(q), RoPE(k)
    out = softmax(q k^T / 8) v   (no mask)
    return concat_heads(out) @ W_o + b_o

Sharding: 8 cores -> (batch b = core//4, head-quad g = core%4, heads 4g..4g+3).
Each core computes partial = attn(heads of g, batch b) @ W_o[rows of g]
for its batch; host sums the 4 partials per batch and adds the bias terms
(b_o and the V-bias contribution b_v @ W_o; softmax rows sum to 1 so the
V bias contributes exactly b_v @ W_o per token).

Device pipeline per core (all matmul operands bf16, fp32 PSUM accum).
ScalarE is the wall: softmax needs 4 heads x 2048 x 2048 = 16.8M exps/core
at 1 elem/lane/cycle @1.2GHz ~= 110us. Everything else is arranged to hide
under that stream:
  - emission is software-pipelined across engines (strict per-engine FIFO):
    K-proj stripe 0 + Q-proj stripe 0 first, then remaining K stripes, then
    the attention kt-loop for q-stripe 0 starts immediately, with V/Q
    projections for later stripes interleaved into the attention chunks
    (PE has ~40% slack under the exp stream).
  - RoPE runs on DVE fully in bf16 (2x/4x perf modes).
  - scores: per (kt, head-pair) one [128, 2, 512] fp32 PSUM tile, two
    K=64 matmuls at stationary base-partitions 0/64 (row-tiled, concurrent);
    ONE exp [128, 1024] PSUM->SBUF bf16 (scale=0.125 folds 1/sqrt(dh)).
  - PV: V stationary [128, 64+2] with two ones columns so PV also yields
    the softmax denominator Z (rows 64:66); accumulate over 16 k-tiles.
  - 1/Z = exp(-ln Z) on ScalarE, batched per (qt, hp) over both heads
    (FD=1024); broadcast across partitions by two col-tiled [2,64] matmuls.
  - PSUM uses all 8 banks: tag "sc" ring 2x4KB (score tiles + Z-broadcast),
    tag "pv" ring 2x4KB (PV accumulators + W_o output tiles).
"""

import math
from contextlib import ExitStack

import numpy as np

import concourse.bass as bass
import concourse.mybir as mybir
import concourse.tile as tile
from concourse.bass_utils import run_bass_kernel_spmd

F32 = mybir.dt.float32
BF = mybir.dt.bfloat16
AF = mybir.ActivationFunctionType

B, S, D = 2, 2048, 1024
NH, HD = 16, 64
ROPE_BASE = 10000.0
N_CORES = 8
HPC = 4  # heads per core
DLOC = HPC * HD  # 256 local head dims per core


def _split_sync_waits(nc, maxw=1):
    """This container's walrus rejects >1-2 SyncWaits per instruction
    ("Too many sync wait commands"). Move excess waits onto NoOps."""
    for f in nc.m.functions:
        for blk in f.blocks:
            new_instructions = []
            for ins in blk.instructions:
                si = getattr(ins, "sync_info", None)
                if si is not None and si.on_wait and len(si.on_wait) > maxw:
                    waits = list(si.on_wait)
                    extra, keep = waits[:-maxw], waits[-maxw:]
                    si.on_wait = keep
                    for i in range(0, len(extra), maxw):
                        nop = mybir.InstNoOp(
                            name=nc.get_next_instruction_name(),
                            engine=ins.engine,
                            sync_info=mybir.SyncInfo(
                                on_wait=extra[i : i + maxw], on_update=[]
                            ),
                        )
                        nc.register_instruction(nop, overwrite=True)
                        new_instructions.append(nop)
                new_instructions.append(ins)
            blk.instructions[:] = new_instructions


def build_attention_nc(seq=S, add_qk_bias=False):
    """One SPMD program; per-core data differs only through inputs."""
    nc = bass.Bass()
    NT = seq // 512  # 512-token stripes
    KT = seq // 128  # k tiles
    NCH = D // 128  # contraction chunks over d_model
    CHUNK = 4  # k-tiles per attention chunk (pipelining granule)

    xT = nc.dram_tensor("xT", [D, seq], BF, kind="ExternalInput")
    wq = nc.dram_tensor("wq", [D, DLOC], BF, kind="ExternalInput")
    wk = nc.dram_tensor("wk", [D, DLOC], BF, kind="ExternalInput")
    wv = nc.dram_tensor("wv", [D, DLOC], BF, kind="ExternalInput")
    wo = nc.dram_tensor("wo", [DLOC, D], BF, kind="ExternalInput")
    cosT = nc.dram_tensor("cosT", [HD, seq], BF, kind="ExternalInput")
    sinT = nc.dram_tensor("sinT", [HD, seq], BF, kind="ExternalInput")
    bqk = nc.dram_tensor("bqk", [2, DLOC], F32, kind="ExternalInput")
    out = nc.dram_tensor("out", [seq, D], F32, kind="ExternalOutput")

    mm = nc.tensor.matmul

    with tile.TileContext(nc) as tc, ExitStack() as ctx:
        consts = ctx.enter_context(tc.tile_pool(name="consts", bufs=1))
        # weights as [128, chunk, cols]; row d = c*128 + p.  DMA order is
        # load-bearing: K-stripe-0 work starts ~4us in, so wk + x stripe 0
        # go first; wo is not needed until the first W_o stage.
        wk_sb = consts.tile([128, NCH, DLOC], BF)
        nc.sync.dma_start(out=wk_sb, in_=wk.rearrange("(c p) m -> p c m", p=128))
        wq_sb = consts.tile([128, NCH, DLOC], BF)
        nc.sync.dma_start(out=wq_sb, in_=wq.rearrange("(c p) m -> p c m", p=128))

        # x stripes: all resident (32KB); stripe 0 DMA'd before cos/sin.
        xpool = ctx.enter_context(tc.tile_pool(name="xT", bufs=NT * NCH))
        xts = [[None] * NCH for _ in range(NT)]

        def dma_x_stripe(nt):
            cs = slice(nt * 512, nt * 512 + 512)
            for c in range(NCH):
                xt = xpool.tile([128, 512], BF, tag="xt", name=f"xt{nt}_{c}")
                nc.sync.dma_start(out=xt, in_=xT[c * 128 : (c + 1) * 128, cs])
                xts[nt][c] = xt

        dma_x_stripe(0)

        # cos/sin rows duplicated for the two heads of a pair
        cs_sb = consts.tile([128, seq], BF)
        nc.sync.dma_start(out=cs_sb[0:HD, :], in_=cosT[:])
        nc.sync.dma_start(out=cs_sb[HD:128, :], in_=cosT[:])
        sn_sb = consts.tile([128, seq], BF)
        nc.sync.dma_start(out=sn_sb[0:HD, :], in_=sinT[:])
        nc.sync.dma_start(out=sn_sb[HD:128, :], in_=sinT[:])
        wv_sb = consts.tile([128, NCH, DLOC], BF)
        nc.sync.dma_start(out=wv_sb, in_=wv.rearrange("(c p) m -> p c m", p=128))
        for nt in range(1, NT):
            dma_x_stripe(nt)
        wo_sb = consts.tile([128, 2, D], BF)
        nc.sync.dma_start(out=wo_sb, in_=wo.rearrange("(c p) m -> p c m", p=128))
        # [2, HD] stationary for the 1/Z partition-broadcast: row0=1, row1=0
        ones_sb = consts.tile([2, HD], BF)
        nc.vector.memset(ones_sb, 0.0)
        nc.vector.memset(ones_sb[0:1, :], 1.0)
        if add_qk_bias:
            bqk_sb = consts.tile([128, 2, 2], F32)
            nc.sync.dma_start(
                out=bqk_sb, in_=bqk.rearrange("b (h p) -> p b h", p=128)
            )

        # long-lived activation tensors
        acts = ctx.enter_context(tc.tile_pool(name="acts", bufs=1))
        qtr = acts.tile([128, 2, seq], BF)  # RoPE'd Q^T, head pairs
        ktr = acts.tile([128, 2, seq], BF)
        v_sb = acts.tile([128, KT, HPC, HD + 2], BF)  # V natural + ones cols
        att = acts.tile([128, 2, seq], BF)  # normalized attn out ^T
        qt_raw = acts.tile([128, 2, seq], BF)
        kt_raw = acts.tile([128, 2, seq], BF)
        # both pad cols 1.0: PSUM rows 64,65 then both hold Z, keeping the
        # 2-partition ln/exp finite (row65's ones_sb weight is 0)
        nc.vector.memset(v_sb[:, :, :, HD : HD + 2], 1.0)

        # working pools.  PSUM budget is exactly 16KB/partition: one shared
        # 2-buf ring of 4KB slots (tag "sc": score tiles, Z-broadcast, Q/K/V
        # projection tiles, W_o output tiles — all short-lived) + a 2-buf
        # ring for the two long-lived PV accumulators (tag "pv").
        rope_tmp = ctx.enter_context(tc.tile_pool(name="ropetmp", bufs=2))
        ps_sc = ctx.enter_context(tc.tile_pool(name="ps_sc", bufs=2, space="PSUM"))
        ps_pv = ctx.enter_context(tc.tile_pool(name="ps_pv", bufs=2, space="PSUM"))
        slab = ctx.enter_context(tc.tile_pool(name="slab", bufs=3))
        npool = ctx.enter_context(tc.tile_pool(name="norm", bufs=2))
        opool = ctx.enter_context(tc.tile_pool(name="ostage", bufs=3))

        # ---------------- phase 1 emitters ---------------------------
        def emit_rope(raw, dst, nt):
            """dst[:, hp, cs] = raw*cos + rot(raw)*sin, all bf16 on DVE.
            rot rows (per 64-block): [0:32] = -raw[32:64], [32:64] = +raw[0:32]
            """
            cs = slice(nt * 512, nt * 512 + 512)
            rot = rope_tmp.tile([128, 2, 512], BF, tag="rot", name="rot")
            for base in (0, 64):
                nc.vector.tensor_scalar_mul(
                    rot[base : base + 32, :, :],
                    raw[base + 32 : base + 64, :, cs],
                    -1.0,
                )
                nc.vector.tensor_copy(
                    rot[base + 32 : base + 64, :, :],
                    raw[base : base + 32, :, cs],
                )
            for hp in range(2):
                tmp = rope_tmp.tile([128, 512], BF, tag="tmp", name="tmp")
                nc.vector.tensor_mul(tmp, raw[:, hp, cs], cs_sb[:, cs])
                rs = rope_tmp.tile([128, 512], BF, tag="rs", name="rs")
                nc.vector.tensor_mul(rs, rot[:, hp, :], sn_sb[:, cs])
                nc.vector.tensor_add(dst[:, hp, cs], tmp, rs)

        def emit_qk(nt, w_sb, raw, dst, bias_row):
            """Project one stripe of Q^T or K^T (PE), drain bf16 (ScalarE),
            RoPE (DVE)."""
            cs = slice(nt * 512, nt * 512 + 512)
            for hp in range(2):
                ps = ps_sc.tile([128, 512], F32, tag="sc", name="qkps")
                for c in range(NCH):
                    mm(
                        ps,
                        w_sb[:, c, hp * 128 : hp * 128 + 128],
                        xts[nt][c],
                        start=(c == 0),
                        stop=(c == NCH - 1),
                    )
                nc.scalar.copy(raw[:, hp, cs], ps)
            if add_qk_bias:
                for hp in range(2):
                    nc.vector.tensor_scalar_add(
                        raw[:, hp, cs], raw[:, hp, cs],
                        bqk_sb[:, bias_row, hp : hp + 1],
                    )
            emit_rope(raw, dst, nt)

        def emit_v(nt):
            """V natural for one stripe: out [128 tokens, 256 vcols] -> v_sb."""
            for tt in range(4):
                ps = ps_sc.tile([128, HPC, HD], F32, tag="sc", name="vps")
                for c in range(NCH):
                    mm(
                        ps,
                        xts[nt][c][:, tt * 128 : tt * 128 + 128],
                        wv_sb[:, c, :],
                        start=(c == 0),
                        stop=(c == NCH - 1),
                    )
                kt_idx = nt * 4 + tt
                nc.vector.tensor_copy(v_sb[:, kt_idx, :, 0:HD], ps)

        # ---------------- phase 2 emitters ---------------------------
        pvt = {}  # (qt-parity key not needed; keyed by hp within current qt)

        def emit_chunk(qt, hp, kts):
            """Scores + exp + PV for kt in kts, one head-pair."""
            qs = slice(qt * 512, qt * 512 + 512)
            if kts[0] == 0:
                pvt[hp] = ps_pv.tile(
                    [128, 2, 512], F32, tag="pv", name=f"pv{hp}"
                )
            for kt_idx in kts:
                sc = ps_sc.tile([128, 2, 512], F32, tag="sc", name="sc")
                for h in range(2):
                    hb = h * 64
                    mm(
                        sc[:, h, :],
                        ktr[hb : hb + 64, hp, kt_idx * 128 : kt_idx * 128 + 128],
                        qtr[hb : hb + 64, hp, qs],
                        start=True,
                        stop=True,
                    )
                pt = slab.tile([128, 2, 512], BF, tag="pt", name="pt")
                nc.scalar.activation(pt, sc, AF.Exp, scale=0.125)
                for h in range(2):
                    mm(
                        pvt[hp][0 : HD + 2, h, :],
                        v_sb[:, kt_idx, hp * 2 + h, :],
                        pt[:, h, :],
                        start=(kt_idx == 0),
                        stop=(kt_idx == KT - 1),
                        skip_group_check=True,
                    )

        def emit_norm(qt, hp):
            """1/Z (ScalarE, both heads batched), partition-broadcast (PE),
            normalize into att (DVE)."""
            qs = slice(qt * 512, qt * 512 + 512)
            pv = pvt[hp]
            lnz = npool.tile([2, 2, 512], F32, tag="lnz", name="lnz")
            nc.scalar.activation(lnz, pv[HD : HD + 2, :, :], AF.Ln)
            zrow = npool.tile([2, 2, 512], BF, tag="z", name="zrow")
            nc.scalar.activation(zrow, lnz, AF.Exp, scale=-1.0)
            zbt = ps_sc.tile([128, 512], F32, tag="sc", name="zbt")
            for h in range(2):
                hb = h * 64
                mm(
                    zbt[hb : hb + 64, :],
                    ones_sb[:, :],
                    zrow[0:2, h, :],
                    start=True,
                    stop=True,
                    tile_position=(0, hb),
                )
            zc = npool.tile([128, 512], BF, tag="zc", name="zc")
            nc.vector.tensor_copy(zc, zbt)
            for h in range(2):
                hb = h * 64
                nc.vector.tensor_mul(
                    att[hb : hb + 64, hp, qs], pv[0:HD, h, :], zc[hb : hb + 64, :]
                )

        def emit_wo(qt):
            """Output projection for this 512-token stripe + DMA out."""
            for tt in range(4):
                tok = qt * 512 + tt * 128
                ps = ps_sc.tile([128, 2, 512], F32, tag="sc", name="wops")
                for nh in range(2):
                    for hp in range(2):
                        mm(
                            ps[:, nh, :],
                            att[:, hp, tok : tok + 128],
                            wo_sb[:, hp, nh * 512 : nh * 512 + 512],
                            start=(hp == 0),
                            stop=(hp == 1),
                        )
                o_out = opool.tile([128, 2, 512], F32, tag="oo", name="oo")
                nc.vector.tensor_copy(o_out, ps)
                nc.sync.dma_start(out=out[tok : tok + 128, :], in_=o_out)

        # ---------------- pipelined emission -------------------------
        # K stripe 0 and Q stripe 0 first (the attention stream's critical
        # inputs), remaining K stripes, then attention on q-stripe 0 with
        # V/Q projections for later stripes slotted into its PE slack.
        emit_qk(0, wk_sb, kt_raw, ktr, 1)
        emit_qk(0, wq_sb, qt_raw, qtr, 0)
        for nt in range(1, NT):
            emit_qk(nt, wk_sb, kt_raw, ktr, 1)
        emit_v(0)
        n_chunks = KT // CHUNK
        for c in range(n_chunks):
            kts = list(range(c * CHUNK, (c + 1) * CHUNK))
            for hp in range(2):
                emit_chunk(0, hp, kts)
            if c + 1 < NT:
                emit_v(c + 1)
            if c >= 1:
                emit_qk(c, wq_sb, qt_raw, qtr, 0)
        for hp in range(2):
            emit_norm(0, hp)
        emit_wo(0)
        for qt in range(1, NT):
            for c in range(n_chunks):
                kts = list(range(c * CHUNK, (c + 1) * CHUNK))
                for hp in range(2):
                    emit_chunk(qt, hp, kts)
            for hp in range(2):
                emit_norm(qt, hp)
            emit_wo(qt)

    _split_sync_waits(nc, maxw=1)
    return nc


_NC_CACHE = {}


def _rope_cos_sin(seq):
    inv_freq = 1.0 / (
        ROPE_BASE ** (np.arange(0, HD, 2, dtype=np.float32) / HD)
    )
    pos = np.arange(seq, dtype=np.float32)
    freqs = pos[:, None] * inv_freq[None, :]  # [seq, 32]
    emb = np.concatenate([freqs, freqs], axis=-1)  # [seq, 64]
    return np.cos(emb).astype(np.float32), np.sin(emb).astype(np.float32)


def _bf16(a):
    import ml_dtypes

    return np.ascontiguousarray(np.asarray(a, dtype=np.float32)).astype(
        ml_dtypes.bfloat16
    )


def _build_in_maps(hs, W_qkv, b_qkv):
    bq, bk = b_qkv[:D], b_qkv[D : 2 * D]
    cos, sin = _rope_cos_sin(hs.shape[1])
    cosT = _bf16(cos.T)
    sinT = _bf16(sin.T)
    in_maps = []
    for core in range(N_CORES):
        bb, g = core // 4, core % 4
        cols = slice(g * DLOC, (g + 1) * DLOC)
        in_maps.append(
            {
                "xT": _bf16(hs[bb].T),
                "wq": _bf16(W_qkv[:, :D][:, cols]),
                "wk": _bf16(W_qkv[:, D : 2 * D][:, cols]),
                "wv": _bf16(W_qkv[:, 2 * D :][:, cols]),
                "wo": None,  # filled by caller
                "cosT": cosT,
                "sinT": sinT,
                "bqk": np.stack([bq[cols], bk[cols]]),
            }
        )
    return in_maps


def kernel(hidden_states, W_qkv, b_qkv, W_o, b_o):
    hs = np.asarray(hidden_states, dtype=np.float32)
    W_qkv = np.asarray(W_qkv, dtype=np.float32)
    b_qkv = np.asarray(b_qkv, dtype=np.float32)
    W_o = np.asarray(W_o, dtype=np.float32)
    b_o = np.asarray(b_o, dtype=np.float32)
    b, seq, d = hs.shape

    bq, bk, bv = b_qkv[:D], b_qkv[D : 2 * D], b_qkv[2 * D :]
    add_qk_bias = bool(np.any(bq) or np.any(bk))

    key = (seq, add_qk_bias)
    if key not in _NC_CACHE:
        _NC_CACHE[key] = build_attention_nc(seq, add_qk_bias)
    nc = _NC_CACHE[key]

    in_maps = _build_in_maps(hs, W_qkv, b_qkv)
    for core in range(N_CORES):
        g = core % 4
        cols = slice(g * DLOC, (g + 1) * DLOC)
        in_maps[core]["wo"] = _bf16(W_o[cols, :])

    res = run_bass_kernel_spmd(nc, in_maps, list(range(N_CORES)))
    parts = [res.results[c]["out"] for c in range(N_CORES)]
    outv = np.stack(
        [parts[0] + parts[1] + parts[2] + parts[3],
         parts[4] + parts[5] + parts[6] + parts[7]]
    )
    outv += b_o[None, None, :] + (bv @ W_o)[None, None, :]
    return outv.astype(np.float32)
